# revision 7
# baseline (speedup 1.0000x reference)
"""Bass/Trainium2 kernel for nn_F_Loss_65446711656630.

Strategy (data-parallel over N, 8 cores):
  - Each core gets 8192 rows of hidden [65536, 512] plus a pre-built
    one-hot class matrix (host-side, from batch_ids).
  - On-device: per-class segment sums and sums-of-squares via one-hot
    matmuls on the TensorEngine, accumulated in PSUM across 64 tiles
    of [128, 512].  This is the memory-bound part (128 MiB streamed).
  - Host: combine the 8 cores' [16,512] stats (fp64), then the tiny
    O(C^2 D) pairwise betainc/top-k stage (C=16) on CPU.
"""

import numpy as np

C = 16
D = 512
N = 65536
NCORES = 8
ROWS = N // NCORES          # 8192 rows per core
P = 128                     # SBUF partitions
TILES = ROWS // P           # 64 tiles per core
XMIN, XMAX = 1e-37, 1.0 - 1e-5

_NC_CACHE = {}


def _build_nc(mm_dtype="float32r"):
    """Per-core SPMD program: stats[0:16]=class sums, stats[16:32]=class sumsq."""
    import concourse.tile as tile
    from concourse import bacc, mybir

    f32 = mybir.dt.float32
    mmdt = getattr(mybir.dt, mm_dtype)

    # Bacc (not raw Bass): its compile() pass splits multi-sem waits into
    # event semaphores, which the TRN2 instruction structs require.
    nc = bacc.Bacc("TRN2", target_bir_lowering=False, debug=False,
                   num_devices=NCORES)
    hidden = nc.declare_dram_parameter("hidden", [ROWS, D], f32, isOutput=False)
    onehot = nc.declare_dram_parameter("onehot", [P, TILES * C], f32, isOutput=False)
    sums = nc.declare_dram_parameter("sums", [C, D], f32, isOutput=True)
    sumsq = nc.declare_dram_parameter("sumsq", [C, D], f32, isOutput=True)

    h_view = hidden[:].rearrange("(t p) d -> t p d", p=P)  # [TILES, 128, 512]

    use_r = mm_dtype != "float32"

    with tile.TileContext(nc) as tc:
        with (
            tc.tile_pool(name="oh", bufs=1) as oh_pool,
            # One slot per tile: h-load DMAs then carry no WAR/WAW waits
            # (HW DGE direct descriptors only support a single sync wait).
            tc.tile_pool(name="h", bufs=TILES) as h_pool,
            tc.tile_pool(name="hr", bufs=4) as hr_pool,
            tc.tile_pool(name="sq", bufs=4) as sq_pool,
            tc.tile_pool(name="psum", bufs=1, space="PSUM") as psum_pool,
            tc.tile_pool(name="outp", bufs=1) as out_pool,
        ):
            oh = oh_pool.tile([P, TILES * C], f32, tag="oh")
            nc.sync.dma_start(oh[:], onehot[:])
            if use_r:
                # fp32r matmul operands must come from a compute op that
                # rounds to fp32r (DMA output is rejected by the verifier).
                # All fp32r producers live on the DVE so each matmul needs
                # only one sync wait (HW limits waits per matmul).
                oh_r = oh_pool.tile([P, TILES * C], mmdt, tag="oh_r")
                nc.vector.tensor_copy(out=oh_r[:], in_=oh[:])
                oh = oh_r

            ps_sum = psum_pool.tile([C, D], f32, tag="ps_sum")
            ps_sq = psum_pool.tile([C, D], f32, tag="ps_sq")

            for t in range(TILES):
                h = h_pool.tile([P, D], f32)
                nc.sync.dma_start(h[:], h_view[t])
                sq = sq_pool.tile([P, D], mmdt if use_r else f32)
                nc.vector.tensor_mul(out=sq[:], in0=h[:], in1=h[:])
                if use_r:
                    hr = hr_pool.tile([P, D], mmdt)
                    nc.vector.tensor_copy(out=hr[:], in_=h[:])
                    h = hr

                first, last = (t == 0), (t == TILES - 1)
                lhsT = oh[:, t * C:(t + 1) * C]
                nc.tensor.matmul(ps_sum[:], lhsT=lhsT, rhs=h[:], start=first, stop=last)
                nc.tensor.matmul(ps_sq[:], lhsT=lhsT, rhs=sq[:], start=first, stop=last)

            ot_sum = out_pool.tile([C, D], f32, tag="ot_sum")
            ot_sq = out_pool.tile([C, D], f32, tag="ot_sq")
            nc.vector.tensor_copy(ot_sum[:], ps_sum[:])
            nc.vector.tensor_copy(ot_sq[:], ps_sq[:])
            nc.sync.dma_start(sums[:], ot_sum[:])
            nc.sync.dma_start(sumsq[:], ot_sq[:])
    nc.compile()
    return nc


def _get_nc(mm_dtype="float32r"):
    if mm_dtype not in _NC_CACHE:
        _NC_CACHE[mm_dtype] = _build_nc(mm_dtype)
    return _NC_CACHE[mm_dtype]


def _device_stats(hidden_np, onehot_np, mm_dtype="float32r", **run_kwargs):
    """Run the SPMD kernel; returns (sums[C,D], sumsq[C,D]) as float64, plus raw results."""
    from concourse import bass_utils

    nc = _get_nc(mm_dtype)
    in_maps = []
    for k in range(NCORES):
        rows = slice(k * ROWS, (k + 1) * ROWS)
        oh_k = (
            onehot_np[rows]
            .reshape(TILES, P, C)
            .transpose(1, 0, 2)
            .reshape(P, TILES * C)
        )
        in_maps.append({
            "hidden": np.ascontiguousarray(hidden_np[rows]),
            "onehot": np.ascontiguousarray(oh_k),
        })
    res = bass_utils.run_bass_kernel_spmd(nc, in_maps, list(range(NCORES)), **run_kwargs)
    sums = np.zeros((C, D), dtype=np.float64)
    sumsq = np.zeros((C, D), dtype=np.float64)
    for r in res.results:
        sums += r["sums"].astype(np.float64)
        sumsq += r["sumsq"].astype(np.float64)
    return sums, sumsq, res


def _pairwise_loss(counts, sums, sumsq, d):
    """The tiny O(C^2 D) stage, float64 on host. Mirrors reference.py exactly."""
    from scipy.special import betainc as sp_betainc

    counts = counts.astype(np.float64)
    means = sums / counts[:, None]                                # [C, D]
    withins = sumsq - counts[:, None] * means**2                  # [C, D]
    half_diff = (means[:, None, :] - means[None, :, :]) * 0.5     # [C, C, D]
    pair_counts = counts[:, None] + counts[None, :]               # [C, C]
    pair_between = half_diff * half_diff * pair_counts[:, :, None]
    pair_within = withins[:, None, :] + withins[None, :, :]
    d2 = pair_counts - 2.0
    d2 = np.where(d2 == 0.0, 1e-5, d2)
    with np.errstate(invalid="ignore", divide="ignore"):
        x = pair_between / (pair_between + pair_within)
    x = np.clip(x, XMIN, XMAX)
    b = np.broadcast_to((d2 * 0.5)[:, :, None], x.shape)
    xbetainc = sp_betainc(0.5, b, x)                              # [C, C, D]
    k = int(d)
    top_k = np.partition(xbetainc, D - k, axis=-1)[..., D - k:]   # [C, C, d]
    per_pair = np.sum(np.log(top_k), axis=-1)                     # [C, C]
    mask = np.triu(np.ones((C, C), dtype=bool), k=1)
    total = np.sum(np.where(mask, per_pair, 0.0))
    return -total


def kernel(hidden, batch_ids, d):
    hidden = np.asarray(hidden, dtype=np.float32)
    ids = np.asarray(batch_ids).astype(np.int64)
    assert hidden.shape == (N, D), hidden.shape

    counts = np.bincount(ids, minlength=C).astype(np.float64)
    onehot = (ids[:, None] == np.arange(C)[None, :]).astype(np.float32)  # [N, C]

    sums, sumsq, _ = _device_stats(hidden, onehot)
    total = _pairwise_loss(counts, sums, sumsq, int(np.asarray(d)))
    return np.array(total, dtype=np.float32)
